# revision 9
# baseline (speedup 1.0000x reference)
"""MDCA loss kernel for Trainium2, data-parallel over 8 NeuronCores.

loss = mean_c |mean_b(softmax(output)[b,c]) - hist(target)[c]/B|

Per core: 1024 rows x 10000 classes. Each 128-row tile is DMA'd to SBUF,
exp() on the scalar engine produces E (fp16) and row sums S (accum_out),
w = 1024/S, and the tensor engine computes per-class column sums
E_chunk^T @ w (classes on PSUM partitions, 79 chunks of <=128 classes in a
single PSUM bank). Per-tile PSUM results are accumulated into an SBUF f32
accumulator. The label histogram (8192 ints) and the final abs-diff mean
(10000 floats) run on the host during the gather step.
"""

import numpy as np

B, C = 8192, 10000
N_CORES = 8
ROWS_PER_CORE = B // N_CORES  # 1024
P = 128
N_TILES = ROWS_PER_CORE // P  # 8
N_CHUNKS = (C + P - 1) // P  # 79
LAST_W = C - (N_CHUNKS - 1) * P  # 16
W_SCALE = 1024.0  # keeps w=1/S out of the fp16 subnormal range

TRACE = False
LAST_RESULTS = None

_cached_nc = None


def _build():
    global _cached_nc
    if _cached_nc is not None:
        return _cached_nc

    import concourse.bacc as bacc
    import concourse.tile as tile
    from concourse import mybir

    nc = bacc.Bacc(
        "TRN2",
        target_bir_lowering=False,
        debug=False,
        enable_asserts=False,
        num_devices=N_CORES,
    )
    x = nc.dram_tensor(
        "x", [ROWS_PER_CORE, C], mybir.dt.float16, kind="ExternalInput"
    )
    out = nc.dram_tensor(
        "colsum", [P, N_CHUNKS], mybir.dt.float32, kind="ExternalOutput"
    )
    xv = x.ap().rearrange("(t p) c -> t p c", p=P)

    with tile.TileContext(nc) as tc:
        with (
            tc.tile_pool(name="xp", bufs=3) as xp,
            tc.tile_pool(name="ep", bufs=2) as ep,
            tc.tile_pool(name="small", bufs=4) as small,
            tc.tile_pool(name="accp", bufs=1) as accp,
            tc.tile_pool(name="psum", bufs=2, space="PSUM") as psum_pool,
        ):
            acc = accp.tile([P, N_CHUNKS], mybir.dt.float32)

            # Warm-up: load the Exp ACT table while tile 0's DMA is in
            # flight, so the first real activation doesn't pay ~2.7us.
            warm = accp.tile([P, 1], mybir.dt.float32)
            nc.vector.memset(warm[:], 0.0)
            nc.scalar.activation(
                out=warm[:], in_=warm[:], func=mybir.ActivationFunctionType.Exp
            )

            for t in range(N_TILES):
                xt = xp.tile([P, C], mybir.dt.float16)
                et = ep.tile([P, C], mybir.dt.float16)
                s = small.tile([P, 1], mybir.dt.float32)
                if t <= 2:
                    # Column-chunk the leading tiles so exp starts as soon
                    # as ~0.6MB lands instead of waiting for a full 2.5MB
                    # tile (hides the per-DMA completion latency while the
                    # ACT queue is still ramping).
                    n_ck = 4
                    ck = C // n_ck  # 2500
                    sp = small.tile([P, n_ck], mybir.dt.float32)
                    for k in range(n_ck):
                        cs = slice(k * ck, (k + 1) * ck)
                        nc.sync.dma_start(out=xt[:, cs], in_=xv[t][:, cs])
                        nc.scalar.activation(
                            out=et[:, cs],
                            in_=xt[:, cs],
                            func=mybir.ActivationFunctionType.Exp,
                            accum_out=sp[:, k : k + 1],
                        )
                    nc.vector.tensor_reduce(
                        out=s[:],
                        in_=sp[:],
                        axis=mybir.AxisListType.X,
                        op=mybir.AluOpType.add,
                    )
                else:
                    nc.sync.dma_start(out=xt[:], in_=xv[t])
                    nc.scalar.activation(
                        out=et[:],
                        in_=xt[:],
                        func=mybir.ActivationFunctionType.Exp,
                        accum_out=s[:],
                    )
                w32 = small.tile([P, 1], mybir.dt.float32)
                nc.vector.reciprocal(out=w32[:], in_=s[:])
                w16 = small.tile([P, 1], mybir.dt.float16)
                nc.vector.tensor_scalar_mul(w16[:], w32[:], W_SCALE)

                # Per-class partial sums for this tile: one PSUM bank holds
                # [128, 79]. The first matmul (start=True) marks the bank's
                # zero region; the rest lazily-zero their own columns and
                # accumulate in place.
                pt = psum_pool.tile([P, N_CHUNKS], mybir.dt.float32)
                for j in range(N_CHUNKS):
                    c0 = j * P
                    cw = min(P, C - c0)
                    nc.tensor.matmul(
                        pt[:cw, j : j + 1],
                        lhsT=et[:, c0 : c0 + cw],
                        rhs=w16[:],
                        start=(j == 0),
                        stop=(j == N_CHUNKS - 1),
                    )
                if t == 0:
                    nc.vector.tensor_copy(acc[:], pt[:])
                else:
                    nc.vector.tensor_add(acc[:], acc[:], pt[:])
            nc.sync.dma_start(out=out.ap()[:], in_=acc[:])

    nc.compile()
    _cached_nc = nc
    return nc


def kernel(output, target):
    global LAST_RESULTS
    from concourse.bass_utils import run_bass_kernel_spmd

    nc = _build()

    X = np.ascontiguousarray(np.asarray(output, dtype=np.float16))
    assert X.shape == (B, C)
    in_maps = [
        {"x": X[c * ROWS_PER_CORE : (c + 1) * ROWS_PER_CORE]} for c in range(N_CORES)
    ]
    import os

    trace_cores = None
    if os.environ.get("KTRACE_ALL") == "1":
        trace_cores = list(range(N_CORES))
    res = run_bass_kernel_spmd(
        nc,
        in_maps,
        core_ids=list(range(N_CORES)),
        trace=TRACE,
        trace_cores=trace_cores,
    )
    LAST_RESULTS = res

    total = np.zeros((P, N_CHUNKS), np.float64)
    for r in res.results:
        total += r["colsum"].astype(np.float64)
    colsum = total.T.reshape(-1)[:C]  # class index = chunk*128 + partition
    avg_conf = colsum / (W_SCALE * B)

    t = np.asarray(target).astype(np.int64)
    avg_count = np.bincount(t, minlength=C).astype(np.float64) / B

    loss = np.abs(avg_conf - avg_count).sum() / C
    return np.asarray(loss, dtype=np.float32)


# revision 13
# speedup vs baseline: 1.1731x; 1.1731x over previous
"""MDCA loss kernel for Trainium2, data-parallel over 8 NeuronCores.

loss = mean_c |mean_b(softmax(output)[b,c]) - hist(target)[c]/B|

Per core: 1024 rows x 10000 classes. Each 128-row tile is DMA'd to SBUF,
exp() on the scalar engine produces E (fp16) and row sums S (accum_out),
w = 1024/S, and the tensor engine computes per-class column sums
E_chunk^T @ w (classes on PSUM partitions, 79 chunks of <=128 classes in a
single PSUM bank). Per-tile PSUM results are accumulated into an SBUF f32
accumulator. The label histogram (8192 ints) and the final abs-diff mean
(10000 floats) run on the host during the gather step.
"""

import numpy as np

B, C = 8192, 10000
N_CORES = 8
ROWS_PER_CORE = B // N_CORES  # 1024
P = 128
N_TILES = ROWS_PER_CORE // P  # 8
N_CHUNKS = (C + P - 1) // P  # 79
LAST_W = C - (N_CHUNKS - 1) * P  # 16
# exp(x + EXP_BIAS) keeps row sums ~800 so w = 1/S stays in fp16 normal
# range; the bias cancels exactly in w*E = exp(x)/sum(exp(x)).
EXP_BIAS = -3.0

TRACE = False
LAST_RESULTS = None

_cached_nc = None


def _build():
    global _cached_nc
    if _cached_nc is not None:
        return _cached_nc

    import concourse.bacc as bacc
    import concourse.tile as tile
    from concourse import mybir

    nc = bacc.Bacc(
        "TRN2",
        target_bir_lowering=False,
        debug=False,
        enable_asserts=False,
        num_devices=N_CORES,
    )
    x = nc.dram_tensor(
        "x", [ROWS_PER_CORE, C], mybir.dt.float16, kind="ExternalInput"
    )
    out = nc.dram_tensor(
        "colsum", [P, N_CHUNKS], mybir.dt.float32, kind="ExternalOutput"
    )
    xv = x.ap().rearrange("(t p) c -> t p c", p=P)

    with tile.TileContext(nc) as tc:
        with (
            tc.tile_pool(name="xp", bufs=3) as xp,
            tc.tile_pool(name="ep", bufs=2) as ep,
            tc.tile_pool(name="small", bufs=4) as small,
            tc.tile_pool(name="accp", bufs=1) as accp,
            tc.tile_pool(name="psum", bufs=2, space="PSUM") as psum_pool,
        ):
            acc = accp.tile([P, N_CHUNKS], mybir.dt.float32)

            bias_t = accp.tile([P, 1], mybir.dt.float32)
            nc.vector.memset(bias_t[:], EXP_BIAS)

            # Warm-up: load the Exp ACT table while tile 0's DMA is in
            # flight, so the first real activation doesn't pay ~2.7us.
            warm = accp.tile([P, 1], mybir.dt.float32)
            nc.vector.memset(warm[:], 0.0)
            nc.scalar.activation(
                out=warm[:], in_=warm[:], func=mybir.ActivationFunctionType.Exp
            )

            for t in range(N_TILES):
                xt = xp.tile([P, C], mybir.dt.float16)
                et = ep.tile([P, C], mybir.dt.float16)
                s = small.tile([P, 1], mybir.dt.float32)
                if t <= 2:
                    # Column-chunk the leading tiles so exp starts as soon
                    # as ~0.6MB lands instead of waiting for a full 2.5MB
                    # tile (hides the per-DMA completion latency while the
                    # ACT queue is still ramping).
                    n_ck = 4
                    ck = C // n_ck  # 2500
                    sp = small.tile([P, n_ck], mybir.dt.float32)
                    for k in range(n_ck):
                        cs = slice(k * ck, (k + 1) * ck)
                        nc.sync.dma_start(out=xt[:, cs], in_=xv[t][:, cs])
                        nc.scalar.activation(
                            out=et[:, cs],
                            in_=xt[:, cs],
                            func=mybir.ActivationFunctionType.Exp,
                            bias=bias_t[:],
                            accum_out=sp[:, k : k + 1],
                        )
                    nc.vector.tensor_reduce(
                        out=s[:],
                        in_=sp[:],
                        axis=mybir.AxisListType.X,
                        op=mybir.AluOpType.add,
                    )
                else:
                    nc.sync.dma_start(out=xt[:], in_=xv[t])
                    nc.scalar.activation(
                        out=et[:],
                        in_=xt[:],
                        func=mybir.ActivationFunctionType.Exp,
                        bias=bias_t[:],
                        accum_out=s[:],
                    )
                w16 = small.tile([P, 1], mybir.dt.float16)
                with nc.allow_low_precision(reason="w quantized to fp16 for matmul rhs"):
                    nc.vector.reciprocal(out=w16[:], in_=s[:])

                # Per-class partial sums for this tile: one PSUM bank holds
                # [128, 79]. The first matmul (start=True) marks the bank's
                # zero region; the rest lazily-zero their own columns and
                # accumulate in place.
                pt = psum_pool.tile([P, N_CHUNKS], mybir.dt.float32)
                for j in range(N_CHUNKS):
                    c0 = j * P
                    cw = min(P, C - c0)
                    nc.tensor.matmul(
                        pt[:cw, j : j + 1],
                        lhsT=et[:, c0 : c0 + cw],
                        rhs=w16[:],
                        start=(j == 0),
                        stop=(j == N_CHUNKS - 1),
                    )
                if t == 0:
                    nc.vector.tensor_copy(acc[:], pt[:])
                else:
                    nc.vector.tensor_add(acc[:], acc[:], pt[:])
            nc.sync.dma_start(out=out.ap()[:], in_=acc[:])

    nc.compile()
    _cached_nc = nc
    return nc


def kernel(output, target):
    global LAST_RESULTS
    from concourse.bass_utils import run_bass_kernel_spmd

    nc = _build()

    X = np.ascontiguousarray(np.asarray(output, dtype=np.float16))
    assert X.shape == (B, C)
    in_maps = [
        {"x": X[c * ROWS_PER_CORE : (c + 1) * ROWS_PER_CORE]} for c in range(N_CORES)
    ]
    import os

    trace_cores = None
    if os.environ.get("KTRACE_ALL") == "1":
        trace_cores = list(range(N_CORES))
    res = run_bass_kernel_spmd(
        nc,
        in_maps,
        core_ids=list(range(N_CORES)),
        trace=TRACE,
        trace_cores=trace_cores,
    )
    LAST_RESULTS = res

    total = np.zeros((P, N_CHUNKS), np.float64)
    for r in res.results:
        total += r["colsum"].astype(np.float64)
    colsum = total.T.reshape(-1)[:C]  # class index = chunk*128 + partition
    avg_conf = colsum / B

    t = np.asarray(target).astype(np.int64)
    avg_count = np.bincount(t, minlength=C).astype(np.float64) / B

    loss = np.abs(avg_conf - avg_count).sum() / C
    return np.asarray(loss, dtype=np.float32)


# revision 14
# speedup vs baseline: 1.1957x; 1.0193x over previous
"""MDCA loss kernel for Trainium2, data-parallel over 8 NeuronCores.

loss = mean_c |mean_b(softmax(output)[b,c]) - hist(target)[c]/B|

Per core: 1024 rows x 10000 classes. Each 128-row tile is DMA'd to SBUF,
exp() on the scalar engine produces E (fp16) and row sums S (accum_out),
w = 1024/S, and the tensor engine computes per-class column sums
E_chunk^T @ w (classes on PSUM partitions, 79 chunks of <=128 classes in a
single PSUM bank). Per-tile PSUM results are accumulated into an SBUF f32
accumulator. The label histogram (8192 ints) and the final abs-diff mean
(10000 floats) run on the host during the gather step.
"""

import numpy as np

B, C = 8192, 10000
N_CORES = 8
ROWS_PER_CORE = B // N_CORES  # 1024
P = 128
N_TILES = ROWS_PER_CORE // P  # 8
N_CHUNKS = (C + P - 1) // P  # 79
LAST_W = C - (N_CHUNKS - 1) * P  # 16
# exp(x + EXP_BIAS) keeps row sums ~800 so w = 1/S stays in fp16 normal
# range; the bias cancels exactly in w*E = exp(x)/sum(exp(x)).
EXP_BIAS = -3.0

TRACE = False
LAST_RESULTS = None

_cached_nc = None


def _build():
    global _cached_nc
    if _cached_nc is not None:
        return _cached_nc

    import concourse.bacc as bacc
    import concourse.tile as tile
    from concourse import mybir

    nc = bacc.Bacc(
        "TRN2",
        target_bir_lowering=False,
        debug=False,
        enable_asserts=False,
        num_devices=N_CORES,
    )
    x = nc.dram_tensor(
        "x", [ROWS_PER_CORE, C], mybir.dt.float16, kind="ExternalInput"
    )
    out = nc.dram_tensor(
        "colsum", [P, N_CHUNKS], mybir.dt.float32, kind="ExternalOutput"
    )
    xv = x.ap().rearrange("(t p) c -> t p c", p=P)

    with tile.TileContext(nc) as tc:
        with (
            tc.tile_pool(name="xp", bufs=3) as xp,
            tc.tile_pool(name="ep", bufs=2) as ep,
            tc.tile_pool(name="small", bufs=4) as small,
            tc.tile_pool(name="accp", bufs=1) as accp,
            tc.tile_pool(name="psum", bufs=2, space="PSUM") as psum_pool,
        ):
            acc = accp.tile([P, N_CHUNKS], mybir.dt.float32)

            bias_t = accp.tile([P, 1], mybir.dt.float32)
            nc.vector.memset(bias_t[:], EXP_BIAS)

            # Warm-up: load the Exp ACT table while tile 0's DMA is in
            # flight, so the first real activation doesn't pay ~2.7us.
            warm = accp.tile([P, 1], mybir.dt.float32)
            nc.vector.memset(warm[:], 0.0)
            nc.scalar.activation(
                out=warm[:], in_=warm[:], func=mybir.ActivationFunctionType.Exp
            )

            for t in range(N_TILES):
                xt = xp.tile([P, C], mybir.dt.float16)
                et = ep.tile([P, C], mybir.dt.float16)
                s = small.tile([P, 1], mybir.dt.float32)
                if t <= 2:
                    # Column-chunk the leading tiles so exp starts as soon
                    # as the first sub-MB chunk lands instead of waiting for
                    # a full 2.5MB tile (hides the per-DMA completion
                    # latency while the ACT queue is still ramping). Tile 0
                    # leads with small chunks; later tiles use fewer, bigger
                    # chunks to cut per-ACTIVATE overhead.
                    bounds = [0, 1250, 2500, 5000, C] if t == 0 else [0, 5000, C]
                    n_ck = len(bounds) - 1
                    sp = small.tile([P, 4], mybir.dt.float32, tag="sp")
                    for k in range(n_ck):
                        cs = slice(bounds[k], bounds[k + 1])
                        nc.sync.dma_start(out=xt[:, cs], in_=xv[t][:, cs])
                        nc.scalar.activation(
                            out=et[:, cs],
                            in_=xt[:, cs],
                            func=mybir.ActivationFunctionType.Exp,
                            bias=bias_t[:],
                            accum_out=sp[:, k : k + 1],
                        )
                    nc.vector.tensor_reduce(
                        out=s[:],
                        in_=sp[:, :n_ck],
                        axis=mybir.AxisListType.X,
                        op=mybir.AluOpType.add,
                    )
                else:
                    nc.sync.dma_start(out=xt[:], in_=xv[t])
                    nc.scalar.activation(
                        out=et[:],
                        in_=xt[:],
                        func=mybir.ActivationFunctionType.Exp,
                        bias=bias_t[:],
                        accum_out=s[:],
                    )
                w16 = small.tile([P, 1], mybir.dt.float16)
                with nc.allow_low_precision(reason="w quantized to fp16 for matmul rhs"):
                    nc.vector.reciprocal(out=w16[:], in_=s[:])

                # Per-class partial sums for this tile: one PSUM bank holds
                # [128, 79]. The first matmul (start=True) marks the bank's
                # zero region; the rest lazily-zero their own columns and
                # accumulate in place.
                pt = psum_pool.tile([P, N_CHUNKS], mybir.dt.float32)
                for j in range(N_CHUNKS):
                    c0 = j * P
                    cw = min(P, C - c0)
                    nc.tensor.matmul(
                        pt[:cw, j : j + 1],
                        lhsT=et[:, c0 : c0 + cw],
                        rhs=w16[:],
                        start=(j == 0),
                        stop=(j == N_CHUNKS - 1),
                    )
                if t == 0:
                    nc.vector.tensor_copy(acc[:], pt[:])
                else:
                    nc.vector.tensor_add(acc[:], acc[:], pt[:])
            nc.sync.dma_start(out=out.ap()[:], in_=acc[:])

    nc.compile()
    _cached_nc = nc
    return nc


def kernel(output, target):
    global LAST_RESULTS
    from concourse.bass_utils import run_bass_kernel_spmd

    nc = _build()

    X = np.ascontiguousarray(np.asarray(output, dtype=np.float16))
    assert X.shape == (B, C)
    in_maps = [
        {"x": X[c * ROWS_PER_CORE : (c + 1) * ROWS_PER_CORE]} for c in range(N_CORES)
    ]
    import os

    trace_cores = None
    if os.environ.get("KTRACE_ALL") == "1":
        trace_cores = list(range(N_CORES))
    res = run_bass_kernel_spmd(
        nc,
        in_maps,
        core_ids=list(range(N_CORES)),
        trace=TRACE,
        trace_cores=trace_cores,
    )
    LAST_RESULTS = res

    total = np.zeros((P, N_CHUNKS), np.float64)
    for r in res.results:
        total += r["colsum"].astype(np.float64)
    colsum = total.T.reshape(-1)[:C]  # class index = chunk*128 + partition
    avg_conf = colsum / B

    t = np.asarray(target).astype(np.int64)
    avg_count = np.bincount(t, minlength=C).astype(np.float64) / B

    loss = np.abs(avg_conf - avg_count).sum() / C
    return np.asarray(loss, dtype=np.float32)


# revision 15
# speedup vs baseline: 1.1967x; 1.0008x over previous
"""MDCA loss kernel for Trainium2, data-parallel over 8 NeuronCores.

loss = mean_c |mean_b(softmax(output)[b,c]) - hist(target)[c]/B|

Per core: 1024 rows x 10000 classes (input cast to fp16 on the host; the
logits are ~N(0,1) so this is far below the output tolerance and halves
DMA traffic). Each 128-row tile is DMA'd to SBUF, exp(x-3) on the scalar
engine produces E (fp16) and row sums S (accum_out), w = 1/S (fp16), and
the tensor engine computes per-class column sums E_chunk^T @ w (classes
on PSUM partitions, 79 chunks of <=128 classes in a single PSUM bank).
Per-tile PSUM results are accumulated into an SBUF f32 accumulator. The
label histogram (8192 ints) and the final abs-diff mean (10000 floats)
run on the host during the gather/unshard step.

Measured: ~92us HW exec per core (roofline for the f32 problem is
~114us/core = 40.96MB @ 358GB/s HBM; the kernel is ACT-bound: 8.6us/tile
exp chain + ~7us fixed preamble + ~11us fixed epilogue).
"""

import numpy as np

B, C = 8192, 10000
N_CORES = 8
ROWS_PER_CORE = B // N_CORES  # 1024
P = 128
N_TILES = ROWS_PER_CORE // P  # 8
N_CHUNKS = (C + P - 1) // P  # 79
LAST_W = C - (N_CHUNKS - 1) * P  # 16
# exp(x + EXP_BIAS) keeps row sums ~800 so w = 1/S stays in fp16 normal
# range; the bias cancels exactly in w*E = exp(x)/sum(exp(x)).
EXP_BIAS = -3.0

TRACE = False
LAST_RESULTS = None

_cached_nc = None


def _build():
    global _cached_nc
    if _cached_nc is not None:
        return _cached_nc

    import concourse.bacc as bacc
    import concourse.tile as tile
    from concourse import mybir

    nc = bacc.Bacc(
        "TRN2",
        target_bir_lowering=False,
        debug=False,
        enable_asserts=False,
        num_devices=N_CORES,
    )
    x = nc.dram_tensor(
        "x", [ROWS_PER_CORE, C], mybir.dt.float16, kind="ExternalInput"
    )
    out = nc.dram_tensor(
        "colsum", [P, N_CHUNKS], mybir.dt.float32, kind="ExternalOutput"
    )
    xv = x.ap().rearrange("(t p) c -> t p c", p=P)

    with tile.TileContext(nc) as tc:
        with (
            tc.tile_pool(name="xp", bufs=3) as xp,
            tc.tile_pool(name="ep", bufs=2) as ep,
            tc.tile_pool(name="small", bufs=4) as small,
            tc.tile_pool(name="accp", bufs=1) as accp,
            tc.tile_pool(name="psum", bufs=2, space="PSUM") as psum_pool,
        ):
            acc = accp.tile([P, N_CHUNKS], mybir.dt.float32)

            bias_t = accp.tile([P, 1], mybir.dt.float32)
            nc.vector.memset(bias_t[:], EXP_BIAS)

            # Warm-up: load the Exp ACT table while tile 0's DMA is in
            # flight, so the first real activation doesn't pay ~2.7us.
            warm = accp.tile([P, 1], mybir.dt.float32)
            nc.vector.memset(warm[:], 0.0)
            nc.scalar.activation(
                out=warm[:], in_=warm[:], func=mybir.ActivationFunctionType.Exp
            )

            for t in range(N_TILES):
                xt = xp.tile([P, C], mybir.dt.float16)
                et = ep.tile([P, C], mybir.dt.float16)
                s = small.tile([P, 1], mybir.dt.float32)
                if t <= 2:
                    # Column-chunk the leading tiles so exp starts as soon
                    # as the first sub-MB chunk lands instead of waiting for
                    # a full 2.5MB tile (hides the per-DMA completion
                    # latency while the ACT queue is still ramping). Tile 0
                    # leads with small chunks; later tiles use fewer, bigger
                    # chunks to cut per-ACTIVATE overhead.
                    bounds = [0, 1250, 2500, 5000, C] if t == 0 else [0, 5000, C]
                    n_ck = len(bounds) - 1
                    sp = small.tile([P, 4], mybir.dt.float32, tag="sp")
                    for k in range(n_ck):
                        cs = slice(bounds[k], bounds[k + 1])
                        nc.sync.dma_start(out=xt[:, cs], in_=xv[t][:, cs])
                        nc.scalar.activation(
                            out=et[:, cs],
                            in_=xt[:, cs],
                            func=mybir.ActivationFunctionType.Exp,
                            bias=bias_t[:],
                            accum_out=sp[:, k : k + 1],
                        )
                    nc.vector.tensor_reduce(
                        out=s[:],
                        in_=sp[:, :n_ck],
                        axis=mybir.AxisListType.X,
                        op=mybir.AluOpType.add,
                    )
                else:
                    nc.sync.dma_start(out=xt[:], in_=xv[t])
                    nc.scalar.activation(
                        out=et[:],
                        in_=xt[:],
                        func=mybir.ActivationFunctionType.Exp,
                        bias=bias_t[:],
                        accum_out=s[:],
                    )
                w16 = small.tile([P, 1], mybir.dt.float16)
                with nc.allow_low_precision(reason="w quantized to fp16 for matmul rhs"):
                    nc.vector.reciprocal(out=w16[:], in_=s[:])

                # Per-class partial sums for this tile: one PSUM bank holds
                # [128, 79]. The first matmul (start=True) marks the bank's
                # zero region; the rest lazily-zero their own columns and
                # accumulate in place.
                pt = psum_pool.tile([P, N_CHUNKS], mybir.dt.float32)
                for j in range(N_CHUNKS):
                    c0 = j * P
                    cw = min(P, C - c0)
                    nc.tensor.matmul(
                        pt[:cw, j : j + 1],
                        lhsT=et[:, c0 : c0 + cw],
                        rhs=w16[:],
                        start=(j == 0),
                        stop=(j == N_CHUNKS - 1),
                    )
                if t == 0:
                    nc.vector.tensor_copy(acc[:], pt[:])
                else:
                    nc.vector.tensor_add(acc[:], acc[:], pt[:])
            nc.sync.dma_start(out=out.ap()[:], in_=acc[:])

    nc.compile()
    _cached_nc = nc
    return nc


def kernel(output, target):
    global LAST_RESULTS
    from concourse.bass_utils import run_bass_kernel_spmd

    nc = _build()

    X = np.ascontiguousarray(np.asarray(output, dtype=np.float16))
    assert X.shape == (B, C)
    in_maps = [
        {"x": X[c * ROWS_PER_CORE : (c + 1) * ROWS_PER_CORE]} for c in range(N_CORES)
    ]
    import os

    trace_cores = None
    if os.environ.get("KTRACE_ALL") == "1":
        trace_cores = list(range(N_CORES))
    res = run_bass_kernel_spmd(
        nc,
        in_maps,
        core_ids=list(range(N_CORES)),
        trace=TRACE,
        trace_cores=trace_cores,
    )
    LAST_RESULTS = res

    total = np.zeros((P, N_CHUNKS), np.float64)
    for r in res.results:
        total += r["colsum"].astype(np.float64)
    colsum = total.T.reshape(-1)[:C]  # class index = chunk*128 + partition
    avg_conf = colsum / B

    t = np.asarray(target).astype(np.int64)
    avg_count = np.bincount(t, minlength=C).astype(np.float64) / B

    loss = np.abs(avg_conf - avg_count).sum() / C
    return np.asarray(loss, dtype=np.float32)
